# revision 3
# baseline (speedup 1.0000x reference)
"""MixerDiffAttention Trainium2 kernel, v2.

Full inputs in, full output out. Shards across 8 NeuronCores:
core c -> batch b = c//4, head-pairs {2j, 2j+1} with j = c%4
(data parallel on B=2, tensor parallel on the 8 v-groups/head-pairs).

v2 design (vs v1): single fused pass over x (x DMA'd once, both pairs'
projections per t-tile), fp16 matmul operands (full PE rate, half DMA/SBUF),
q/k transposed to [d, t] via the DMA xbar engine (PE transposes and PSUM
evacuation copies eliminated), rope + rms-scale fused into few wide DVE ops
via stride-0 broadcast APs, attention probabilities in bf16 (exp range needs
fp32-exponent), denominator from an augmented ones-column in V. Phases are
issued serially so the activation table switches exactly once (Sqrt -> Exp).

Per-core layout:
  heads hh = 0..3 = (pair, a): (0,0),(0,1),(1,0),(1,1); global q/k head
  h = 2j + pair + 8a; v-group g = 2j + pair (256 cols at out col pair*256).
"""

import math
import sys

_TRN = "/opt/trn_rl_repo"
if _TRN not in sys.path:
    sys.path.insert(0, _TRN)

import numpy as np

import concourse.bass as bass
import concourse.mybir as mybir
import concourse.tile as tile
from concourse import bacc
from concourse.bass import broadcast_tensor_aps
from concourse.bass_utils import run_bass_kernel_spmd

F32 = mybir.dt.float32
F16 = mybir.dt.float16
BF16 = mybir.dt.bfloat16
AF = mybir.ActivationFunctionType
OP = mybir.AluOpType

B, D = 2, 2048
NH, HD = 16, 128
LAMBDA_INIT = 0.8 - 0.6 * math.exp(-0.3 * 0)
EPS = float(np.finfo(np.float32).eps)
P = 128
QC = 256
N_CORES = 8

_CACHE = {}


def _bmul(nc, out, a, b_):
    a2, b2 = broadcast_tensor_aps(a, b_)
    nc.vector.tensor_mul(out, a2, b2)


def build_nc(T, reps=1):
    TT = T // P
    DK = D // P
    NQC = T // QC

    nc = bacc.Bacc("TRN2", target_bir_lowering=False, debug=False)

    xt_d = nc.dram_tensor("xt", [P, TT, DK, P], F16, kind="ExternalInput")
    wqk_d = nc.dram_tensor("wqk", [P, DK, 1024], F16, kind="ExternalInput")
    wv_d = nc.dram_tensor("wv", [P, DK, 512], F16, kind="ExternalInput")
    ra_d = nc.dram_tensor("ropea", [P, TT, 128], F16, kind="ExternalInput")
    rb_d = nc.dram_tensor("ropeb", [P, TT, 128], F16, kind="ExternalInput")
    qsc_d = nc.dram_tensor("qsc", [P, TT, 4], F32, kind="ExternalInput")
    mask_d = nc.dram_tensor("masks", [P, 512], BF16, kind="ExternalInput")
    nlam_d = nc.dram_tensor("nlam", [P, 1], F32, kind="ExternalInput")
    out_d = nc.dram_tensor("out", [T, 512], F32, kind="ExternalOutput")

    with tile.TileContext(nc) as tc:
        with (
            tc.tile_pool(name="const", bufs=1) as constp,
            tc.tile_pool(name="wpool", bufs=1) as wpool,
            tc.tile_pool(name="xcol", bufs=4) as xcolp,
            tc.tile_pool(name="qkv", bufs=1) as qkvp,
            tc.tile_pool(name="work", bufs=3) as work,
            tc.tile_pool(name="qnp", bufs=6) as qnp,
            tc.tile_pool(name="yst", bufs=3) as ystp,
            tc.tile_pool(name="pt", bufs=4) as ptp,
            tc.tile_pool(name="pp", bufs=4, space="PSUM") as pp,
            tc.tile_pool(name="pa", bufs=4, space="PSUM") as pa,
        ):
            # ---- constants / weights (issue x tile 0 first for fast start) --
            xcol0 = xcolp.tile([P, DK, P], F16, tag="xcol")
            nc.sync.dma_start(xcol0[:, 0:4], xt_d[:, 0, 0:4])
            nc.sync.dma_start(xcol0[:, 4:DK], xt_d[:, 0, 4:DK])

            wqk = wpool.tile([P, DK, 1024], F16, tag="wqk")
            wv = wpool.tile([P, DK, 512], F16, tag="wv")
            # weights + constants go on the Activation DMA queue so the
            # x-column / transpose stream on SP is never blocked behind them
            for dk2 in range(0, DK, 2):
                nc.scalar.dma_start(
                    wqk[:, dk2:dk2 + 2], wqk_d[:, dk2:dk2 + 2]
                )
                nc.scalar.dma_start(wv[:, dk2:dk2 + 2], wv_d[:, dk2:dk2 + 2])

            ra = constp.tile([P, TT, 128], F16)
            rb = constp.tile([P, TT, 128], F16)
            qscb = constp.tile([P, TT, 4], F32)
            maskb = constp.tile([P, 512], BF16)
            nlamb = constp.tile([P, 1], F32)
            epsb = constp.tile([P, 1], F32)
            nc.scalar.dma_start(ra[:], ra_d[:])
            nc.scalar.dma_start(rb[:], rb_d[:])
            nc.scalar.dma_start(qscb[:], qsc_d[:])
            nc.scalar.dma_start(maskb[:], mask_d[:])
            nc.scalar.dma_start(nlamb[:], nlam_d[:])
            nc.vector.memset(epsb[:], EPS)

            qT = qkvp.tile([P, 4, T], F16, tag="qT")
            kT = qkvp.tile([P, 4, T], F16, tag="kT")
            vaug = [
                qkvp.tile([P, TT, 258], BF16, tag=f"vaug{pair}",
                          name=f"vaug{pair}")
                for pair in range(2)
            ]
            for pair in range(2):
                nc.vector.memset(vaug[pair][:, :, 256:258], 1.0)

            for _rep in range(reps):
                # ---- phase A: project, rms-stats, rope, scale, transpose ----
                for tt in range(TT):
                    if tt == 0 and _rep == 0:
                        xcol = xcol0
                    else:
                        xcol = xcolp.tile([P, DK, P], F16, tag="xcol")
                        nc.sync.dma_start(xcol[:], xt_d[:, tt])

                    q_ps = pp.tile([P, 512], F32, tag="pp", name="q_ps")
                    k_ps = pp.tile([P, 512], F32, tag="pp", name="k_ps")
                    v_ps = pp.tile([P, 512], F32, tag="pp", name="v_ps")
                    for dk in range(DK):
                        st = xcol[:, dk, :]
                        nc.tensor.matmul(
                            q_ps[:], st, wqk[:, dk, 0:512],
                            start=(dk == 0), stop=(dk == DK - 1),
                        )
                        nc.tensor.matmul(
                            k_ps[:], st, wqk[:, dk, 512:1024],
                            start=(dk == 0), stop=(dk == DK - 1),
                        )
                        nc.tensor.matmul(
                            v_ps[:], st, wv[:, dk, :],
                            start=(dk == 0), stop=(dk == DK - 1),
                        )

                    # evacuate V first (frees its bank earliest)
                    for pair in range(2):
                        nc.scalar.copy(
                            vaug[pair][:, tt, 0:256],
                            v_ps[:, pair * 256:(pair + 1) * 256],
                        )
                    qk_sb = work.tile([P, 1024], F16, tag="qk_sb")
                    nc.scalar.copy(qk_sb[:, 0:512], q_ps[:])
                    nc.scalar.copy(qk_sb[:, 512:1024], k_ps[:])

                    # sum of squares per head (rotation preserves norms)
                    sq = work.tile([P, 1024], F16, tag="sq")
                    nc.vector.tensor_mul(sq[:], qk_sb[:], qk_sb[:])
                    ms = work.tile([P, 8], F32, tag="ms")
                    nc.vector.reduce_sum(
                        ms[:],
                        sq[:].rearrange("p (g d) -> p g d", d=P),
                        axis=mybir.AxisListType.X,
                    )
                    sqs = work.tile([P, 8], F32, tag="sqs")
                    nc.scalar.activation(
                        sqs[:], ms[:], AF.Sqrt, bias=epsb[:], scale=1.0 / HD
                    )
                    rstd = work.tile([P, 8], F32, tag="rstd")
                    nc.vector.reciprocal(rstd[:], sqs[:])
                    # sc8 = [rstd_q * qscale | rstd_k], fp16
                    sc8 = work.tile([P, 8], F16, tag="sc8")
                    nc.vector.tensor_mul(
                        sc8[:, 0:4], rstd[:, 0:4], qscb[:, tt]
                    )
                    nc.vector.tensor_copy(sc8[:, 4:8], rstd[:, 4:8])

                    # rope: tmp1 = x (.) [cos|sin], tmp2 = x (.) [sin|cos]
                    qv = qk_sb[:].rearrange("p (g d) -> p g d", d=P)
                    tmp1 = work.tile([P, 1024], F16, tag="tmp1")
                    tmp2 = work.tile([P, 1024], F16, tag="tmp2")
                    t1v = tmp1[:].rearrange("p (g d) -> p g d", d=P)
                    t2v = tmp2[:].rearrange("p (g d) -> p g d", d=P)
                    _bmul(nc, t1v, qv, ra[:, tt:tt + 1, :])
                    _bmul(nc, t2v, qv, rb[:, tt:tt + 1, :])
                    rot = work.tile([P, 1024], F16, tag="rot")
                    rv = rot[:].rearrange("p (g two d) -> p g two d", two=2, d=64)
                    t1h = tmp1[:].rearrange("p (g two d) -> p g two d", two=2, d=64)
                    t2h = tmp2[:].rearrange("p (g two d) -> p g two d", two=2, d=64)
                    nc.vector.tensor_add(
                        rv[:, :, 0, :], t1h[:, :, 0, :], t1h[:, :, 1, :]
                    )
                    nc.vector.tensor_sub(
                        rv[:, :, 1, :], t2h[:, :, 1, :], t2h[:, :, 0, :]
                    )

                    # normalize + scale all 8 heads in one op
                    qn = qnp.tile([P, 1024], F16, tag="qn")
                    _bmul(
                        nc,
                        qn[:].rearrange("p (g d) -> p g d", d=P),
                        rot[:].rearrange("p (g d) -> p g d", d=P),
                        sc8[:].rearrange("p (g one) -> p g one", one=1),
                    )

                    # transpose [t, d] -> [d, t] on the DMA xbar
                    for g in range(8):
                        dst = qT if g < 4 else kT
                        hh = g % 4
                        nc.sync.dma_start_transpose(
                            dst[:, hh, tt * P:(tt + 1) * P],
                            qn[:, g * P:(g + 1) * P],
                        )

                # ---- phase C: causal attention ----
                # qc descending (long chains first, short ones fill the tail);
                # the two difference-heads (a=0/1) interleave so PE never
                # waits on a single exp chain.
                # first block needs only early-tt transposes (qc=3), last
                # blocks are short chains so the pipeline drains quickly
                big = [3, 5, NQC - 1, NQC - 2, 4, 2] if NQC == 8 else \
                    list(range(NQC - 1, 1, -1))
                big = [q for q in big if 2 <= q < NQC]
                small = [1, 0] if NQC > 1 else [0]
                sched = [(p, q) for p in range(2) for q in big] + \
                        [(p, q) for p in range(2) for q in small]
                for pair, qc in sched:
                    if True:
                        nkt = 2 * qc + 2
                        y1s = ystp.tile([P, 2, 256], F32, tag="y1s")
                        y_ps = [
                            pa.tile([P, 258], F32, tag="pa", name=f"y{a}{qt}")
                            for a in range(2) for qt in range(2)
                        ]
                        for kt2 in range(nkt // 2):
                            diag = kt2 == nkt // 2 - 1
                            for a in range(2):
                                hh = pair * 2 + a
                                s2 = pp.tile([P, 512], F32, tag="pp",
                                             name="s2")
                                qb = qc * QC
                                kt0 = 2 * kt2
                                nc.tensor.matmul(
                                    s2[:, 0:256],
                                    kT[:, hh, kt0 * P:(kt0 + 1) * P],
                                    qT[:, hh, qb:qb + QC],
                                    start=True, stop=True,
                                )
                                if diag:
                                    # odd diagonal tile: only q-cols 128:256
                                    # are unmasked
                                    nc.tensor.matmul(
                                        s2[:, 256:384],
                                        kT[:, hh, (kt0 + 1) * P:(kt0 + 2) * P],
                                        qT[:, hh, qb + P:qb + QC],
                                        start=True, stop=True,
                                    )
                                    pt = ptp.tile([P, 512], BF16, tag="pt")
                                    nc.scalar.activation(
                                        pt[:, 0:384], s2[:, 0:384], AF.Exp
                                    )
                                    pm = ptp.tile([P, 512], BF16, tag="pt")
                                    nc.vector.tensor_mul(
                                        pm[:, 0:384], pt[:, 0:384],
                                        maskb[:, 0:384]
                                    )
                                    # (kt0, qt0) is the final contribution
                                    # for qt0; (kt0+1, qt0) is fully masked
                                    nc.tensor.matmul(
                                        y_ps[2 * a][:], pm[:, 0:128],
                                        vaug[pair][:, kt0, :],
                                        start=(kt0 == 0), stop=True,
                                    )
                                    nc.tensor.matmul(
                                        y_ps[2 * a + 1][:], pm[:, 128:256],
                                        vaug[pair][:, kt0, :],
                                        start=(kt0 == 0), stop=False,
                                    )
                                    nc.tensor.matmul(
                                        y_ps[2 * a + 1][:], pm[:, 256:384],
                                        vaug[pair][:, kt0 + 1, :],
                                        start=False, stop=True,
                                    )
                                else:
                                    nc.tensor.matmul(
                                        s2[:, 256:512],
                                        kT[:, hh, (kt0 + 1) * P:(kt0 + 2) * P],
                                        qT[:, hh, qb:qb + QC],
                                        start=True, stop=True,
                                    )
                                    pt = ptp.tile([P, 512], BF16, tag="pt")
                                    nc.scalar.activation(pt[:], s2[:], AF.Exp)
                                    for half in range(2):
                                        kt = kt0 + half
                                        for qt in range(2):
                                            nc.tensor.matmul(
                                                y_ps[2 * a + qt][:],
                                                pt[:, half * 256 + qt * P:
                                                   half * 256 + (qt + 1) * P],
                                                vaug[pair][:, kt, :],
                                                start=(kt == 0),
                                                stop=False,
                                            )
                        yo = ystp.tile([P, 2, 256], F32, tag="yo")
                        for a in range(2):
                            for qt in range(2):
                                rz = work.tile([P, 1], F32, tag="rz")
                                nc.vector.reciprocal(
                                    rz[:], y_ps[2 * a + qt][:, 256:257]
                                )
                                if a == 0:
                                    nc.vector.tensor_scalar_mul(
                                        y1s[:, qt, :],
                                        y_ps[qt][:, 0:256], rz[:]
                                    )
                                else:
                                    nlr = work.tile([P, 1], F32, tag="nlr")
                                    nc.vector.tensor_mul(
                                        nlr[:], rz[:], nlamb[:]
                                    )
                                    nc.vector.scalar_tensor_tensor(
                                        yo[:, qt, :], y_ps[2 + qt][:, 0:256],
                                        nlr[:], y1s[:, qt, :],
                                        op0=OP.mult, op1=OP.add,
                                    )
                        nc.sync.dma_start(
                            out_d[qc * QC:(qc + 1) * QC,
                                  pair * 256:(pair + 1) * 256]
                            .rearrange("(qt p) c -> p qt c", qt=2),
                            yo[:],
                        )

    nc.compile()
    return nc


# ---------------- host-side prep ----------------

def _rotary_tables(T):
    inv_freq = (
        1.0 / (10000.0 ** (np.arange(0, HD, 2, dtype=np.float32) / np.float32(HD)))
    ).astype(np.float32)
    freqs = np.arange(T, dtype=np.float32)[:, None] * inv_freq[None, :]
    f64 = freqs.astype(np.float64)
    return np.cos(f64).astype(np.float32), np.sin(f64).astype(np.float32)


def prepare_in_maps(x, Wq, Wk, Wv, lambda_q1, lambda_k1, lambda_q2, lambda_k2,
                    softmax_scaler, T):
    TT, DK = T // P, D // P
    lam_full = float(
        np.exp(np.sum(lambda_q1.astype(np.float64) * lambda_k1.astype(np.float64)))
        - np.exp(np.sum(lambda_q2.astype(np.float64) * lambda_k2.astype(np.float64)))
        + LAMBDA_INIT
    )
    cos, sin = _rotary_tables(T)
    log_pos = np.log(np.arange(1, T + 1, dtype=np.float32)).astype(np.float32)
    sc = softmax_scaler.reshape(NH).astype(np.float32)
    qhead_scale = (log_pos[:, None] * sc[None, :] / np.float32(math.sqrt(HD)))

    # shared across cores
    ropea = np.concatenate([cos, sin], axis=1)  # [T, 128]
    ropeb = np.concatenate([sin, cos], axis=1)
    ropea = np.ascontiguousarray(
        ropea.reshape(TT, P, 128).transpose(1, 0, 2)).astype(np.float16)
    ropeb = np.ascontiguousarray(
        ropeb.reshape(TT, P, 128).transpose(1, 0, 2)).astype(np.float16)

    bf16 = mybir.dt.np(BF16)
    i = np.arange(P)[:, None]
    jj = np.arange(256)[None, :]
    m0 = (i <= jj).astype(np.float32)
    masks = np.concatenate([m0, m0[:, 0:128], np.zeros((P, 128), np.float32)],
                           axis=1).astype(bf16)

    nlam = np.full((P, 1), -lam_full, np.float32)

    # per-batch x arrangement (shared by the 4 cores of that batch)
    xts = []
    for b in range(B):
        xa = x[b].reshape(TT, P, DK, P).transpose(3, 0, 2, 1)
        xts.append(np.ascontiguousarray(xa, dtype=np.float16))

    in_maps = []
    for c in range(N_CORES):
        b, j = c // 4, c % 4
        heads = [2 * j, 2 * j + 8, 2 * j + 1, 2 * j + 9]  # hh = 0..3
        wq_c = np.concatenate(
            [Wq[:, h * HD:(h + 1) * HD] for h in heads], axis=1)
        wk_c = np.concatenate(
            [Wk[:, h * HD:(h + 1) * HD] for h in heads], axis=1)
        wqk = np.concatenate([wq_c, wk_c], axis=1)  # [D, 1024]
        wqk = np.ascontiguousarray(
            wqk.reshape(DK, P, 1024).transpose(1, 0, 2), dtype=np.float16)
        g0 = 2 * j
        wv_c = np.ascontiguousarray(
            Wv[:, g0 * 256:(g0 + 2) * 256].reshape(DK, P, 512).transpose(1, 0, 2),
            dtype=np.float16)

        qs = np.stack([qhead_scale[:, h] for h in heads], axis=1)  # [T, 4]
        qsb = np.ascontiguousarray(
            qs.reshape(TT, P, 4).transpose(1, 0, 2)).astype(np.float32)

        in_maps.append({
            "xt": xts[b],
            "wqk": wqk,
            "wv": wv_c,
            "ropea": ropea,
            "ropeb": ropeb,
            "qsc": qsb,
            "masks": masks,
            "nlam": nlam,
        })
    return in_maps


def kernel(x, Wq, Wk, Wv, lambda_q1, lambda_k1, lambda_q2, lambda_k2,
           softmax_scaler):
    T = x.shape[1]
    in_maps = prepare_in_maps(
        np.asarray(x, np.float32), np.asarray(Wq, np.float32),
        np.asarray(Wk, np.float32), np.asarray(Wv, np.float32),
        np.asarray(lambda_q1, np.float32), np.asarray(lambda_k1, np.float32),
        np.asarray(lambda_q2, np.float32), np.asarray(lambda_k2, np.float32),
        np.asarray(softmax_scaler, np.float32), T,
    )
    if T not in _CACHE:
        _CACHE[T] = build_nc(T)
    nc = _CACHE[T]
    res = run_bass_kernel_spmd(nc, in_maps, list(range(N_CORES)))
    out = np.empty((B, T, D), np.float32)
    for c in range(N_CORES):
        b, j = c // 4, c % 4
        out[b, :, 512 * j:512 * (j + 1)] = res.results[c]["out"]
    return out


# revision 4
# speedup vs baseline: 1.1586x; 1.1586x over previous
"""MixerDiffAttention Trainium2 kernel, v2.

Full inputs in, full output out. Shards across 8 NeuronCores:
core c -> batch b = c//4, head-pairs {2j, 2j+1} with j = c%4
(data parallel on B=2, tensor parallel on the 8 v-groups/head-pairs).

v2 design (vs v1): single fused pass over x (x DMA'd once, both pairs'
projections per t-tile), fp16 matmul operands (full PE rate, half DMA/SBUF),
q/k transposed to [d, t] via the DMA xbar engine (PE transposes and PSUM
evacuation copies eliminated), rope + rms-scale fused into few wide DVE ops
via stride-0 broadcast APs, attention probabilities in bf16 (exp range needs
fp32-exponent), denominator from an augmented ones-column in V. Phases are
issued serially so the activation table switches exactly once (Sqrt -> Exp).

Per-core layout:
  heads hh = 0..3 = (pair, a): (0,0),(0,1),(1,0),(1,1); global q/k head
  h = 2j + pair + 8a; v-group g = 2j + pair (256 cols at out col pair*256).
"""

import math
import sys

_TRN = "/opt/trn_rl_repo"
if _TRN not in sys.path:
    sys.path.insert(0, _TRN)

import numpy as np

import concourse.bass as bass
import concourse.mybir as mybir
import concourse.tile as tile
from concourse import bacc
from concourse.bass import broadcast_tensor_aps
from concourse.bass_utils import run_bass_kernel_spmd

F32 = mybir.dt.float32
F16 = mybir.dt.float16
BF16 = mybir.dt.bfloat16
AF = mybir.ActivationFunctionType
OP = mybir.AluOpType

B, D = 2, 2048
NH, HD = 16, 128
LAMBDA_INIT = 0.8 - 0.6 * math.exp(-0.3 * 0)
EPS = float(np.finfo(np.float32).eps)
P = 128
QC = 256
N_CORES = 8

_CACHE = {}


def _bmul(nc, out, a, b_):
    a2, b2 = broadcast_tensor_aps(a, b_)
    nc.vector.tensor_mul(out, a2, b2)


def build_nc(T, reps=1):
    TT = T // P
    DK = D // P
    NQC = T // QC

    nc = bacc.Bacc("TRN2", target_bir_lowering=False, debug=False)

    xt_d = nc.dram_tensor("xt", [P, TT, DK, P], F16, kind="ExternalInput")
    wqk_d = nc.dram_tensor("wqk", [P, DK, 1024], F16, kind="ExternalInput")
    wv_d = nc.dram_tensor("wv", [P, DK, 512], F16, kind="ExternalInput")
    ra_d = nc.dram_tensor("ropea", [P, TT, 128], F16, kind="ExternalInput")
    rb_d = nc.dram_tensor("ropeb", [P, TT, 128], F16, kind="ExternalInput")
    qsc_d = nc.dram_tensor("qsc", [P, TT, 4], F32, kind="ExternalInput")
    mask_d = nc.dram_tensor("masks", [P, 512], BF16, kind="ExternalInput")
    nlam_d = nc.dram_tensor("nlam", [P, 1], F32, kind="ExternalInput")
    out_d = nc.dram_tensor("out", [T, 512], F32, kind="ExternalOutput")

    with tile.TileContext(nc) as tc:
        with (
            tc.tile_pool(name="const", bufs=1) as constp,
            tc.tile_pool(name="wpool", bufs=1) as wpool,
            tc.tile_pool(name="xcol", bufs=4) as xcolp,
            tc.tile_pool(name="qkv", bufs=1) as qkvp,
            tc.tile_pool(name="work", bufs=3) as work,
            tc.tile_pool(name="qnp", bufs=6) as qnp,
            tc.tile_pool(name="yst", bufs=3) as ystp,
            tc.tile_pool(name="pt", bufs=4) as ptp,
            tc.tile_pool(name="pp", bufs=4, space="PSUM") as pp,
            tc.tile_pool(name="pa", bufs=4, space="PSUM") as pa,
        ):
            # ---- constants / weights (issue x tile 0 first for fast start) --
            xcol0 = xcolp.tile([P, DK, P], F16, tag="xcol")
            nc.sync.dma_start(xcol0[:, 0:4], xt_d[:, 0, 0:4])
            nc.sync.dma_start(xcol0[:, 4:DK], xt_d[:, 0, 4:DK])

            wqk = wpool.tile([P, DK, 1024], F16, tag="wqk")
            wv = wpool.tile([P, DK, 512], F16, tag="wv")
            # weights + constants go on the Activation DMA queue so the
            # x-column / transpose stream on SP is never blocked behind them
            for dk2 in range(0, DK, 2):
                nc.scalar.dma_start(
                    wqk[:, dk2:dk2 + 2], wqk_d[:, dk2:dk2 + 2]
                )
                nc.scalar.dma_start(wv[:, dk2:dk2 + 2], wv_d[:, dk2:dk2 + 2])

            ra = constp.tile([P, TT, 128], F16)
            rb = constp.tile([P, TT, 128], F16)
            qscb = constp.tile([P, TT, 4], F32)
            maskb = constp.tile([P, 512], BF16)
            nlamb = constp.tile([P, 1], F32)
            epsb = constp.tile([P, 1], F32)
            nc.scalar.dma_start(ra[:], ra_d[:])
            nc.scalar.dma_start(rb[:], rb_d[:])
            nc.scalar.dma_start(qscb[:], qsc_d[:])
            nc.scalar.dma_start(maskb[:], mask_d[:])
            nc.scalar.dma_start(nlamb[:], nlam_d[:])
            nc.vector.memset(epsb[:], EPS)

            qT = qkvp.tile([P, 4, T], F16, tag="qT")
            kT = qkvp.tile([P, 4, T], F16, tag="kT")
            vaug2 = qkvp.tile([P, TT, 2, 258], BF16, tag="vaug2")
            nc.vector.memset(vaug2[:, :, :, 256:258], 1.0)

            for _rep in range(reps):
                # ---- phase A: project, rms-stats, rope, scale, transpose ----
                for tt in range(TT):
                    if tt == 0 and _rep == 0:
                        xcol = xcol0
                    else:
                        xcol = xcolp.tile([P, DK, P], F16, tag="xcol")
                        nc.sync.dma_start(xcol[:], xt_d[:, tt])

                    q_ps = pp.tile([P, 512], F32, tag="pp", name="q_ps")
                    k_ps = pp.tile([P, 512], F32, tag="pp", name="k_ps")
                    v_ps = pp.tile([P, 512], F32, tag="pp", name="v_ps")
                    for dk in range(DK):
                        st = xcol[:, dk, :]
                        nc.tensor.matmul(
                            q_ps[:], st, wqk[:, dk, 0:512],
                            start=(dk == 0), stop=(dk == DK - 1),
                        )
                        nc.tensor.matmul(
                            k_ps[:], st, wqk[:, dk, 512:1024],
                            start=(dk == 0), stop=(dk == DK - 1),
                        )
                        nc.tensor.matmul(
                            v_ps[:], st, wv[:, dk, :],
                            start=(dk == 0), stop=(dk == DK - 1),
                        )

                    # evacuate V first (frees its bank earliest); both
                    # pairs in one strided copy
                    nc.scalar.copy(
                        vaug2[:, tt, :, 0:256],
                        v_ps[:].rearrange("p (a c) -> p a c", a=2),
                    )
                    qk_sb = work.tile([P, 1024], F16, tag="qk_sb")
                    nc.scalar.copy(qk_sb[:, 0:512], q_ps[:])
                    nc.scalar.copy(qk_sb[:, 512:1024], k_ps[:])

                    # sum of squares per head (rotation preserves norms)
                    sq = work.tile([P, 1024], F16, tag="sq")
                    nc.vector.tensor_mul(sq[:], qk_sb[:], qk_sb[:])
                    ms = work.tile([P, 8], F32, tag="ms")
                    nc.vector.reduce_sum(
                        ms[:],
                        sq[:].rearrange("p (g d) -> p g d", d=P),
                        axis=mybir.AxisListType.X,
                    )
                    # rstd = rsqrt(ms/HD + eps) via bit-trick + 2 Newton
                    # iterations, all on DVE: keeps the ACT table on Exp for
                    # the whole kernel (no Sqrt<->Exp table reloads)
                    xms = work.tile([P, 8], F32, tag="xms")
                    nc.vector.tensor_scalar(
                        xms[:], ms[:], 1.0 / HD, EPS,
                        op0=OP.mult, op1=OP.add,
                    )
                    y0i = work.tile([P, 8], mybir.dt.int32, tag="y0i")
                    nc.vector.tensor_scalar(
                        y0i[:], xms[:].bitcast(mybir.dt.int32), 1, None,
                        op0=OP.logical_shift_right,
                    )
                    rstd = work.tile([P, 8], F32, tag="rstd")
                    nc.vector.tensor_scalar(
                        rstd[:].bitcast(mybir.dt.int32), y0i[:],
                        -1, 0x5F3759DF, op0=OP.mult, op1=OP.add,
                    )
                    for _it in range(2):
                        yy = work.tile([P, 8], F32, tag="yy")
                        nc.vector.tensor_mul(yy[:], rstd[:], rstd[:])
                        nc.vector.tensor_mul(yy[:], yy[:], xms[:])
                        nc.vector.tensor_scalar(
                            yy[:], yy[:], -0.5, 1.5, op0=OP.mult, op1=OP.add
                        )
                        nc.vector.tensor_mul(rstd[:], rstd[:], yy[:])
                    # sc8 = [rstd_q * qscale | rstd_k], fp16
                    sc8 = work.tile([P, 8], F16, tag="sc8")
                    nc.vector.tensor_mul(
                        sc8[:, 0:4], rstd[:, 0:4], qscb[:, tt]
                    )
                    nc.vector.tensor_copy(sc8[:, 4:8], rstd[:, 4:8])

                    # rope: tmp1 = x (.) [cos|sin], tmp2 = x (.) [sin|cos]
                    qv = qk_sb[:].rearrange("p (g d) -> p g d", d=P)
                    tmp1 = work.tile([P, 1024], F16, tag="tmp1")
                    tmp2 = work.tile([P, 1024], F16, tag="tmp2")
                    t1v = tmp1[:].rearrange("p (g d) -> p g d", d=P)
                    t2v = tmp2[:].rearrange("p (g d) -> p g d", d=P)
                    _bmul(nc, t1v, qv, ra[:, tt:tt + 1, :])
                    _bmul(nc, t2v, qv, rb[:, tt:tt + 1, :])
                    rot = work.tile([P, 1024], F16, tag="rot")
                    rv = rot[:].rearrange("p (g two d) -> p g two d", two=2, d=64)
                    t1h = tmp1[:].rearrange("p (g two d) -> p g two d", two=2, d=64)
                    t2h = tmp2[:].rearrange("p (g two d) -> p g two d", two=2, d=64)
                    nc.vector.tensor_add(
                        rv[:, :, 0, :], t1h[:, :, 0, :], t1h[:, :, 1, :]
                    )
                    nc.vector.tensor_sub(
                        rv[:, :, 1, :], t2h[:, :, 1, :], t2h[:, :, 0, :]
                    )

                    # normalize + scale all 8 heads in one op
                    qn = qnp.tile([P, 1024], F16, tag="qn")
                    _bmul(
                        nc,
                        qn[:].rearrange("p (g d) -> p g d", d=P),
                        rot[:].rearrange("p (g d) -> p g d", d=P),
                        sc8[:].rearrange("p (g one) -> p g one", one=1),
                    )

                    # transpose [t, d] -> [d, t] on the DMA xbar
                    for g in range(8):
                        dst = qT if g < 4 else kT
                        hh = g % 4
                        nc.sync.dma_start_transpose(
                            dst[:, hh, tt * P:(tt + 1) * P],
                            qn[:, g * P:(g + 1) * P],
                        )

                # ---- phase C: causal attention ----
                # qc descending (long chains first, short ones fill the tail);
                # the two difference-heads (a=0/1) interleave so PE never
                # waits on a single exp chain.
                # first block needs only early-tt transposes (qc=3), last
                # blocks are short chains so the pipeline drains quickly
                if NQC == 8:
                    sched = [(0, 3), (0, 5), (0, 7), (0, 6), (0, 4), (0, 2),
                             (1, 3), (0, 1), (1, 5), (0, 0), (1, 7), (1, 6),
                             (1, 4), (1, 2), (1, 1), (1, 0)]
                else:
                    big = [q for q in range(NQC - 1, 1, -1)]
                    small = [1, 0] if NQC > 1 else [0]
                    sched = [(p, q) for p in range(2) for q in big] + \
                            [(p, q) for p in range(2) for q in small]
                for pair, qc in sched:
                    if True:
                        nkt = 2 * qc + 2
                        y1s = ystp.tile([P, 2, 256], F32, tag="y1s")
                        y_ps = [
                            pa.tile([P, 258], F32, tag="pa", name=f"y{a}{qt}")
                            for a in range(2) for qt in range(2)
                        ]
                        def issue_s(kt2):
                            dg = kt2 == nkt // 2 - 1
                            kt0 = 2 * kt2
                            qb = qc * QC
                            tiles = []
                            for a in range(2):
                                hh = pair * 2 + a
                                s2 = pp.tile([P, 512], F32, tag="pp",
                                             name="s2")
                                nc.tensor.matmul(
                                    s2[:, 0:256],
                                    kT[:, hh, kt0 * P:(kt0 + 1) * P],
                                    qT[:, hh, qb:qb + QC],
                                    start=True, stop=True,
                                )
                                if dg:
                                    # odd diagonal tile: only q-cols 128:256
                                    # are unmasked
                                    nc.tensor.matmul(
                                        s2[:, 256:384],
                                        kT[:, hh, (kt0 + 1) * P:(kt0 + 2) * P],
                                        qT[:, hh, qb + P:qb + QC],
                                        start=True, stop=True,
                                    )
                                else:
                                    nc.tensor.matmul(
                                        s2[:, 256:512],
                                        kT[:, hh, (kt0 + 1) * P:(kt0 + 2) * P],
                                        qT[:, hh, qb:qb + QC],
                                        start=True, stop=True,
                                    )
                                tiles.append(s2)
                            return tiles

                        # S matmuls for kt2+1 are issued before the PV
                        # matmuls of kt2 so the in-order PE queue always has
                        # independent work while exp(kt2) runs
                        s2_cur = issue_s(0)
                        for kt2 in range(nkt // 2):
                            diag = kt2 == nkt // 2 - 1
                            kt0 = 2 * kt2
                            s2_next = (issue_s(kt2 + 1)
                                       if kt2 + 1 < nkt // 2 else None)
                            for a in range(2):
                                s2 = s2_cur[a]
                                if diag:
                                    pt = ptp.tile([P, 512], BF16, tag="pt")
                                    nc.scalar.activation(
                                        pt[:, 0:384], s2[:, 0:384], AF.Exp
                                    )
                                    pm = ptp.tile([P, 512], BF16, tag="pt")
                                    nc.vector.tensor_mul(
                                        pm[:, 0:384], pt[:, 0:384],
                                        maskb[:, 0:384]
                                    )
                                    # (kt0, qt0) is the final contribution
                                    # for qt0; (kt0+1, qt0) is fully masked
                                    nc.tensor.matmul(
                                        y_ps[2 * a][:], pm[:, 0:128],
                                        vaug2[:, kt0, pair, :],
                                        start=(kt0 == 0), stop=True,
                                    )
                                    nc.tensor.matmul(
                                        y_ps[2 * a + 1][:], pm[:, 128:256],
                                        vaug2[:, kt0, pair, :],
                                        start=(kt0 == 0), stop=False,
                                    )
                                    nc.tensor.matmul(
                                        y_ps[2 * a + 1][:], pm[:, 256:384],
                                        vaug2[:, kt0 + 1, pair, :],
                                        start=False, stop=True,
                                    )
                                else:
                                    pt = ptp.tile([P, 512], BF16, tag="pt")
                                    nc.scalar.activation(pt[:], s2[:], AF.Exp)
                                    for half in range(2):
                                        kt = kt0 + half
                                        for qt in range(2):
                                            nc.tensor.matmul(
                                                y_ps[2 * a + qt][:],
                                                pt[:, half * 256 + qt * P:
                                                   half * 256 + (qt + 1) * P],
                                                vaug2[:, kt, pair, :],
                                                start=(kt == 0),
                                                stop=False,
                                            )
                            s2_cur = s2_next
                        yo = ystp.tile([P, 2, 256], F32, tag="yo")
                        for a in range(2):
                            for qt in range(2):
                                rz = work.tile([P, 1], F32, tag="rz")
                                nc.vector.reciprocal(
                                    rz[:], y_ps[2 * a + qt][:, 256:257]
                                )
                                if a == 0:
                                    nc.vector.tensor_scalar_mul(
                                        y1s[:, qt, :],
                                        y_ps[qt][:, 0:256], rz[:]
                                    )
                                else:
                                    nlr = work.tile([P, 1], F32, tag="nlr")
                                    nc.vector.tensor_mul(
                                        nlr[:], rz[:], nlamb[:]
                                    )
                                    nc.vector.scalar_tensor_tensor(
                                        yo[:, qt, :], y_ps[2 + qt][:, 0:256],
                                        nlr[:], y1s[:, qt, :],
                                        op0=OP.mult, op1=OP.add,
                                    )
                        nc.sync.dma_start(
                            out_d[qc * QC:(qc + 1) * QC,
                                  pair * 256:(pair + 1) * 256]
                            .rearrange("(qt p) c -> p qt c", qt=2),
                            yo[:],
                        )

    nc.compile()
    return nc


# ---------------- host-side prep ----------------

def _rotary_tables(T):
    inv_freq = (
        1.0 / (10000.0 ** (np.arange(0, HD, 2, dtype=np.float32) / np.float32(HD)))
    ).astype(np.float32)
    freqs = np.arange(T, dtype=np.float32)[:, None] * inv_freq[None, :]
    f64 = freqs.astype(np.float64)
    return np.cos(f64).astype(np.float32), np.sin(f64).astype(np.float32)


def prepare_in_maps(x, Wq, Wk, Wv, lambda_q1, lambda_k1, lambda_q2, lambda_k2,
                    softmax_scaler, T):
    TT, DK = T // P, D // P
    lam_full = float(
        np.exp(np.sum(lambda_q1.astype(np.float64) * lambda_k1.astype(np.float64)))
        - np.exp(np.sum(lambda_q2.astype(np.float64) * lambda_k2.astype(np.float64)))
        + LAMBDA_INIT
    )
    cos, sin = _rotary_tables(T)
    log_pos = np.log(np.arange(1, T + 1, dtype=np.float32)).astype(np.float32)
    sc = softmax_scaler.reshape(NH).astype(np.float32)
    qhead_scale = (log_pos[:, None] * sc[None, :] / np.float32(math.sqrt(HD)))

    # shared across cores
    ropea = np.concatenate([cos, sin], axis=1)  # [T, 128]
    ropeb = np.concatenate([sin, cos], axis=1)
    ropea = np.ascontiguousarray(
        ropea.reshape(TT, P, 128).transpose(1, 0, 2)).astype(np.float16)
    ropeb = np.ascontiguousarray(
        ropeb.reshape(TT, P, 128).transpose(1, 0, 2)).astype(np.float16)

    bf16 = mybir.dt.np(BF16)
    i = np.arange(P)[:, None]
    jj = np.arange(256)[None, :]
    m0 = (i <= jj).astype(np.float32)
    masks = np.concatenate([m0, m0[:, 0:128], np.zeros((P, 128), np.float32)],
                           axis=1).astype(bf16)

    nlam = np.full((P, 1), -lam_full, np.float32)

    # per-batch x arrangement (shared by the 4 cores of that batch)
    xts = []
    for b in range(B):
        xa = x[b].reshape(TT, P, DK, P).transpose(3, 0, 2, 1)
        xts.append(np.ascontiguousarray(xa, dtype=np.float16))

    in_maps = []
    for c in range(N_CORES):
        b, j = c // 4, c % 4
        heads = [2 * j, 2 * j + 8, 2 * j + 1, 2 * j + 9]  # hh = 0..3
        wq_c = np.concatenate(
            [Wq[:, h * HD:(h + 1) * HD] for h in heads], axis=1)
        wk_c = np.concatenate(
            [Wk[:, h * HD:(h + 1) * HD] for h in heads], axis=1)
        wqk = np.concatenate([wq_c, wk_c], axis=1)  # [D, 1024]
        wqk = np.ascontiguousarray(
            wqk.reshape(DK, P, 1024).transpose(1, 0, 2), dtype=np.float16)
        g0 = 2 * j
        wv_c = np.ascontiguousarray(
            Wv[:, g0 * 256:(g0 + 2) * 256].reshape(DK, P, 512).transpose(1, 0, 2),
            dtype=np.float16)

        qs = np.stack([qhead_scale[:, h] for h in heads], axis=1)  # [T, 4]
        qsb = np.ascontiguousarray(
            qs.reshape(TT, P, 4).transpose(1, 0, 2)).astype(np.float32)

        in_maps.append({
            "xt": xts[b],
            "wqk": wqk,
            "wv": wv_c,
            "ropea": ropea,
            "ropeb": ropeb,
            "qsc": qsb,
            "masks": masks,
            "nlam": nlam,
        })
    return in_maps


def kernel(x, Wq, Wk, Wv, lambda_q1, lambda_k1, lambda_q2, lambda_k2,
           softmax_scaler):
    T = x.shape[1]
    in_maps = prepare_in_maps(
        np.asarray(x, np.float32), np.asarray(Wq, np.float32),
        np.asarray(Wk, np.float32), np.asarray(Wv, np.float32),
        np.asarray(lambda_q1, np.float32), np.asarray(lambda_k1, np.float32),
        np.asarray(lambda_q2, np.float32), np.asarray(lambda_k2, np.float32),
        np.asarray(softmax_scaler, np.float32), T,
    )
    if T not in _CACHE:
        _CACHE[T] = build_nc(T)
    nc = _CACHE[T]
    res = run_bass_kernel_spmd(nc, in_maps, list(range(N_CORES)))
    out = np.empty((B, T, D), np.float32)
    for c in range(N_CORES):
        b, j = c // 4, c % 4
        out[b, :, 512 * j:512 * (j + 1)] = res.results[c]["out"]
    return out
